# revision 45
# baseline (speedup 1.0000x reference)
import numpy as np

import bass_rust
import concourse.bass as bass
import concourse.tile as tile
import concourse.mybir as mybir
from concourse.bass_utils import run_bass_kernel_spmd

B, S, D = 2, 2048, 2048
NH, NKV, HD = 16, 4, 128
GQ = 512
NKO = D // 128
PC = 512
NPC = S // PC
QC = 512
NQC = S // QC
MAGIC = float(np.float32(12582912.0))
SM_SCALE = 1.0 / float(np.sqrt(HD))

F32 = mybir.dt.float32
F32R = mybir.dt.float32r
BF16 = mybir.dt.bfloat16
MULT = mybir.AluOpType.mult
ADD = mybir.AluOpType.add
EXP = mybir.ActivationFunctionType.Exp

_CACHE = {}

LAST_RESULTS = None


def _split_multi_waits(nc):
    for f in nc.m.functions:
        for bb in f.blocks:
            new = []
            for inst in bb.instructions:
                si = inst.sync_info
                if si is None:
                    new.append(inst)
                    continue
                waits = list(si.on_wait)
                if len(waits) > 1:
                    for k, w in enumerate(waits[:-1]):
                        nop = mybir.InstNoOp(name=f"{inst.name}-w{k}", ins=[], outs=[])
                        nop.engine = inst.engine
                        nop.sync_info = bass_rust.SyncInfo(on_wait=[w], on_update=[])
                        new.append(nop)
                    inst.sync_info = bass_rust.SyncInfo(
                        on_wait=[waits[-1]], on_update=list(si.on_update)
                    )
                new.append(inst)
            bb.instructions = new


def _host_consts():
    theta = 10000.0
    angles = 1.0 / theta ** (np.arange(0, HD, 2, dtype=np.float32) / HD)
    emb = np.outer(np.arange(S, dtype=np.float32), angles)
    emb = np.concatenate([emb, emb], axis=-1)
    cos = np.cos(emb).astype(np.float32)
    sin = np.sin(emb).astype(np.float32)
    cosT = np.ascontiguousarray(cos.T)
    sinT = np.ascontiguousarray(sin.T)

    ctd = np.ascontiguousarray(cos.reshape(S // 128, 128, HD).transpose(1, 0, 2))
    std = sin.reshape(S // 128, 128, HD).transpose(1, 0, 2).copy()
    sgn = std.copy()
    sgn[:, :, : HD // 2] = -std[:, :, : HD // 2]
    sgn = np.ascontiguousarray(sgn)

    rot = np.zeros((128, 128), dtype=np.float32)
    for i in range(64):
        rot[i, i + 64] = 1.0
        rot[i + 64, i] = -1.0

    p = np.arange(128)[:, None]
    f = np.arange(128)[None, :]
    tril = (p <= f).astype(np.float32)
    m3 = np.concatenate([np.zeros((128, 128), np.float32), tril], axis=1)

    ones = np.ones((128, 128), dtype=np.float32)
    ident = np.eye(128, dtype=np.float32)
    import ml_dtypes
    bf16 = ml_dtypes.bfloat16
    return {
        "cosT": cosT.astype(bf16), "sinT": sinT.astype(bf16),
        "ctd": ctd.astype(bf16), "sgn": sgn.astype(bf16),
        "rot": rot, "tril": tril, "m3": m3, "ones": ones, "ident": ident,
    }


def _build_nc():
    nc = bass.Bass("TRN2", target_bir_lowering=False, debug=False)

    dataT = nc.dram_tensor("dataT", [D, S], BF16, kind="ExternalInput").ap()
    wq = nc.dram_tensor("wq", [D, GQ], BF16, kind="ExternalInput").ap()
    wkv = nc.dram_tensor("wkv", [D, 2 * HD], BF16, kind="ExternalInput").ap()
    wo = nc.dram_tensor("wo", [GQ, D], BF16, kind="ExternalInput").ap()
    cosT_d = nc.dram_tensor("cosT", [128, S], BF16, kind="ExternalInput").ap()
    sinT_d = nc.dram_tensor("sinT", [128, S], BF16, kind="ExternalInput").ap()
    ctd_d = nc.dram_tensor("ctd", [128, NKO, HD], BF16, kind="ExternalInput").ap()
    sgn_d = nc.dram_tensor("sgn", [128, NKO, HD], BF16, kind="ExternalInput").ap()
    rot_d = nc.dram_tensor("rot", [128, 128], F32R, kind="ExternalInput").ap()
    tril_d = nc.dram_tensor("tril", [128, 128], F32R, kind="ExternalInput").ap()
    m3_d = nc.dram_tensor("m3", [128, 256], F32R, kind="ExternalInput").ap()
    ones_d = nc.dram_tensor("ones", [128, 128], F32R, kind="ExternalInput").ap()
    ident_d = nc.dram_tensor("ident", [128, 128], F32R, kind="ExternalInput").ap()
    outT = nc.dram_tensor("outT", [D, S], BF16, kind="ExternalOutput").ap()

    dataT_r = dataT.rearrange("(ko p) t -> p ko t", p=128)
    wq_r = wq.rearrange("(ko p) m -> p ko m", p=128)
    wkv_r = wkv.rearrange("(ko p) m -> p ko m", p=128)
    wo_r = wo.rearrange("(h p) n -> p h n", p=128)

    from contextlib import ExitStack
    with tile.TileContext(nc) as tc, ExitStack() as stack:
        small_consts = stack.enter_context(tc.tile_pool(name="sconsts", bufs=1))
        rot_sb = small_consts.tile([128, 128], F32R)
        ones_sb = small_consts.tile([128, 128], F32R)
        id_sb = small_consts.tile([128, 128], F32R)
        tril_sb = small_consts.tile([128, 128], F32R)
        m3_sb = small_consts.tile([128, 256], F32R)

        persist = stack.enter_context(tc.tile_pool(name="persist", bufs=1))
        xq4 = persist.tile([128, 4, S], F32R, name="xq4")
        kt4 = persist.tile([128, 4, QC], F32R, name="kt4")
        v_g = [persist.tile([128, 4, HD], F32R, tag=f"vg{g}", name=f"v_g{g}")
               for g in range(4)]

        GRP = 4

        with tc.tile_pool(name="p1consts", bufs=1) as p1c, \
             tc.tile_pool(name="wpool", bufs=1) as wpool, \
             tc.tile_pool(name="datapool", bufs=2) as datapool, \
             tc.tile_pool(name="kvstage", bufs=2) as kvstage, \
             tc.tile_pool(name="qtmp", bufs=2) as qtmp, \
             tc.tile_pool(name="t2pool", bufs=3) as t2pool, \
             tc.tile_pool(name="proj_ps", bufs=3, space="PSUM") as proj_ps, \
             tc.tile_pool(name="kv_ps", bufs=2, space="PSUM") as kv_ps, \
             tc.tile_pool(name="rope_ps", bufs=2, space="PSUM") as rope_ps, \
             tc.tile_pool(name="tp_ps", bufs=1, space="PSUM") as tp_ps:
            cos_sb = p1c.tile([128, S], BF16)
            sin_sb = p1c.tile([128, S], BF16)
            ctd_sb = p1c.tile([128, NKO, HD], BF16)
            sgn_sb = p1c.tile([128, NKO, HD], BF16)
            wq_sb = wpool.tile([128, NKO, GQ], BF16)
            wkv_sb = wpool.tile([128, NKO, 2 * HD], BF16)

            dT = {}
            for c in range(2):
                dT[c] = datapool.tile([128, NKO, PC], BF16, tag="dT",
                                      name=f"dT{c}")

            nc.gpsimd.dma_start(wkv_sb[:], wkv_r[:])
            nc.sync.dma_start(dT[0][:, 0:8], dataT_r[:, 0:8, bass.ts(0, PC)])
            nc.gpsimd.dma_start(dT[0][:, 8:16], dataT_r[:, 8:16, bass.ts(0, PC)])
            nc.sync.dma_start(wq_sb[:], wq_r[:])
            nc.gpsimd.dma_start(dT[1][:, 8:16], dataT_r[:, 8:16, bass.ts(1, PC)])
            nc.sync.dma_start(dT[1][:, 0:8], dataT_r[:, 0:8, bass.ts(1, PC)])
            nc.sync.dma_start(ctd_sb[:], ctd_d[:])
            nc.sync.dma_start(sgn_sb[:], sgn_d[:])
            nc.sync.dma_start(cos_sb[:], cosT_d[:])
            nc.sync.dma_start(sin_sb[:], sinT_d[:])
            nc.sync.dma_start(rot_sb[:], rot_d[:])
            nc.sync.dma_start(ones_sb[:], ones_d[:])
            nc.sync.dma_start(id_sb[:], ident_d[:])
            nc.sync.dma_start(tril_sb[:], tril_d[:])
            nc.sync.dma_start(m3_sb[:], m3_d[:])

            warm = wpool.tile([128, QC], F32, name="warm_scratch")
            nc.vector.memset(warm[:], 0.0)

            def warm_fill(n, w=QC):
                for _ in range(n):
                    wps = rope_ps.tile([128, QC], F32, tag="pr")
                    nc.tensor.matmul(wps[:, 0:w], warm[:, 0:128], warm[:, 0:w],
                                     start=True, stop=True)

            warm_fill(10)

            def quant_group(src_ap, dst_ap):
                amax = qtmp.tile([128, GRP, 1], F32, tag="amax")
                scl = qtmp.tile([128, GRP, 1], F32, tag="scl")
                inv = qtmp.tile([128, GRP, 1], F32, tag="inv")
                xs = qtmp.tile([128, GRP, HD], F32, tag="xs")
                nc.vector.tensor_reduce(amax[:], src_ap, mybir.AxisListType.X,
                                        mybir.AluOpType.max,
                                        apply_absolute_value=True)
                nc.vector.tensor_scalar_max(amax[:], amax[:], 1e-8)
                nc.vector.tensor_scalar_mul(scl[:], amax[:], 1.0 / 127.0)
                nc.vector.reciprocal(inv[:], scl[:])
                sclb = scl[:].to_broadcast((128, GRP, HD))
                invb = inv[:].to_broadcast((128, GRP, HD))
                nc.vector.tensor_tensor(xs[:], src_ap, invb, MULT)
                nc.vector.tensor_scalar_add(xs[:], xs[:], MAGIC)
                nc.vector.tensor_scalar_add(xs[:], xs[:], -MAGIC)
                nc.vector.tensor_tensor(dst_ap, xs[:], sclb, MULT)

            for c in range(NPC):
                csl = bass.ts(c, PC)
                if c + 2 < NPC:
                    cb = c + 2
                    t_ = datapool.tile([128, NKO, PC], BF16, tag="dT",
                                       name=f"dT{cb}")
                    dT[cb] = t_
                    eng = nc.gpsimd if cb % 2 else nc.sync
                    eng.dma_start(t_[:, 0:8], dataT_r[:, 0:8, bass.ts(cb, PC)])
                    eng.dma_start(t_[:, 8:16], dataT_r[:, 8:16, bass.ts(cb, PC)])

                kv_td = kvstage.tile([128, GRP, 2 * HD], F32, tag="kvtd",
                                     name=f"kvtd{c}")
                for j in range(GRP):
                    pkv = kv_ps.tile([128, 2 * HD], F32, tag="pkv")
                    for ko in range(NKO):
                        nc.tensor.matmul(pkv[:],
                                         dT[c][:, ko, bass.ds(j * 128, 128)],
                                         wkv_sb[:, ko],
                                         start=(ko == 0), stop=(ko == NKO - 1))
                    nc.scalar.copy(kv_td[:, j, :], pkv[:])

                kr = kvstage.tile([128, GRP, HD], F32, tag="kr", name=f"kr{c}")
                t2k = qtmp.tile([128, GRP, HD], F32, tag="t2k")
                tsl = bass.ts(c, GRP)
                nc.vector.tensor_tensor(kr[:], kv_td[:, :, 0:HD],
                                        ctd_sb[:, tsl], MULT)
                nc.vector.tensor_tensor(t2k[:, :, 0:64],
                                        kv_td[:, :, 64:HD],
                                        sgn_sb[:, tsl, 0:64], MULT)
                nc.vector.tensor_tensor(t2k[:, :, 64:HD],
                                        kv_td[:, :, 0:64],
                                        sgn_sb[:, tsl, 64:HD], MULT)
                nc.vector.tensor_tensor(kr[:], kr[:], t2k[:], ADD)

                kq = kvstage.tile([128, GRP, HD], F32R, tag="kq", name=f"kq{c}")
                quant_group(kr[:], kq[:])
                if c < NPC - 1:
                    quant_group(kv_td[:, :, HD:], v_g[c][:])

                def emit_qproj(h):
                    pq = proj_ps.tile([128, QC], F32, tag="pq",
                                      name=f"pq{c}_{h}")
                    for ko in range(NKO):
                        nc.tensor.matmul(pq[:], wq_sb[:, ko, bass.ts(h, 128)],
                                         dT[c][:, ko],
                                         start=(ko == 0), stop=(ko == NKO - 1))
                    nc.scalar.copy(xq4[:, h, csl], pq[:])

                emit_qproj(0)
                emit_qproj(1)
                for j in range(GRP):
                    pt = tp_ps.tile([128, 128], F32R, tag="tp")
                    nc.tensor.transpose(pt[:], kq[:, j, :], id_sb[:])
                    nc.scalar.copy(kt4[:, c, bass.ts(j, 128)], pt[:])
                emit_qproj(2)
                emit_qproj(3)
                for h in range(4):
                    pr = rope_ps.tile([128, QC], F32, tag="pr")
                    nc.tensor.matmul(pr[:], rot_sb[:], xq4[:, h, csl],
                                     start=True, stop=True)
                    t1 = t2pool.tile([128, QC], F32, tag="t1")
                    t2 = t2pool.tile([128, QC], F32, tag="t2")
                    nc.vector.tensor_tensor(t1[:], xq4[:, h, csl],
                                            cos_sb[:, csl], MULT)
                    nc.vector.tensor_tensor(t2[:], pr[:], sin_sb[:, csl], MULT)
                    nc.vector.tensor_tensor(xq4[:, h, csl], t1[:], t2[:], ADD)
                if c == NPC - 1:
                    quant_group(kv_td[:, :, HD:], v_g[c][:])

        with tc.tile_pool(name="p2", bufs=1) as p2, \
             tc.tile_pool(name="attn_sb", bufs=8) as attn_sb, \
             tc.tile_pool(name="exp_pool", bufs=14) as exp_pool, \
             tc.tile_pool(name="araw", bufs=6) as araw_pool, \
             tc.tile_pool(name="pssq", bufs=3) as pssq_pool, \
             tc.tile_pool(name="rc4p", bufs=2) as rc_pool, \
             tc.tile_pool(name="outstage", bufs=4) as outstage, \
             tc.tile_pool(name="score_ps", bufs=4, space="PSUM") as score_ps, \
             tc.tile_pool(name="attn_ps", bufs=2, space="PSUM") as attn_ps, \
             tc.tile_pool(name="sum_ps", bufs=2, space="PSUM") as sum_ps:
            out_ps = score_ps
            wo_t = [p2.tile([128, S], BF16, tag=f"wo{h}", name=f"wo{h}")
                    for h in range(4)]
            for h in range(4):
                nc.sync.dma_start(wo_t[h][:], wo_r[:, h])

            def out_proj_unit(c_prev, tiles, dt_):
                cpsl = bass.ts(c_prev, QC)
                po = out_ps.tile([128, QC], F32, tag="ps")
                for h2 in range(4):
                    nc.tensor.matmul(po[:], wo_t[h2][:, bass.ts(dt_, 128)],
                                     tiles[h2][:],
                                     start=(h2 == 0), stop=(h2 == 3))
                ot = outstage.tile([128, QC], BF16, tag="ot")
                nc.vector.tensor_copy(ot[:], po[:])
                eng = nc.gpsimd if dt_ % 2 else nc.sync
                eng.dma_start(outT[bass.ts(dt_, 128), cpsl], ot[:])

            def out_proj(c_prev, tiles):
                cpsl = bass.ts(c_prev, QC)
                engs = [nc.sync, nc.gpsimd, nc.scalar]
                pos = {}
                pools = [out_ps] * 4 + [attn_ps] * 2 + [sum_ps] * 2

                def finish(dt_):
                    po = pos.pop(dt_)
                    for h2 in (2, 3):
                        nc.tensor.matmul(po[:], wo_t[h2][:, bass.ts(dt_, 128)],
                                         tiles[h2][:],
                                         start=False, stop=(h2 == 3))
                    ot = outstage.tile([128, QC], BF16, tag="ot")
                    nc.vector.tensor_copy(ot[:], po[:])
                    engs[dt_ % 3].dma_start(outT[bass.ts(dt_, 128), cpsl],
                                            ot[:])

                tags = ["ps"] * 4 + ["pa"] * 2 + ["pss"] * 2
                for dt_ in range(NKO):
                    po = pools[dt_ % 8].tile([128, QC], F32, tag=tags[dt_ % 8])
                    pos[dt_] = po
                    for h2 in range(2):
                        nc.tensor.matmul(po[:], wo_t[h2][:, bass.ts(dt_, 128)],
                                         tiles[h2][:],
                                         start=(h2 == 0), stop=False)
                    if dt_ >= 7:
                        finish(dt_ - 7)
                for dt_ in range(NKO - 7, NKO):
                    finish(dt_)

            prev = None

            def emit_pair(c, hA, hB, attn_tiles, carry_in, fillers=()):
                fillers = list(fillers)
                nki = 4 * (c + 1)
                streams = (hA, hB)
                pa = [attn_ps.tile([128, QC], F32, tag="pa",
                                   name=f"pa{c}_{h}") for h in streams]
                pss = [sum_ps.tile([128, QC], F32, tag="pss",
                                   name=f"pss{c}_{h}") for h in streams]

                def emit_acc(st, ki, et, qoff):
                    nc.tensor.matmul(pss[st][:, qoff:], ones_sb[:],
                                     et[:, qoff:],
                                     start=(ki == 0), stop=(ki == nki - 1))
                    nc.tensor.matmul(pa[st][:, qoff:],
                                     v_g[ki // 4][:, ki % 4], et[:, qoff:],
                                     start=(ki == 0), stop=(ki == nki - 1))

                pending = []
                for ki in range(nki):
                    if ki >= 4 * c:
                        j = ki - 4 * c
                        qoff = min(128 * j, 256)
                    else:
                        j, qoff = -1, 0
                    w = QC - qoff
                    for st in range(2):
                        h = streams[st]
                        ps = score_ps.tile([128, QC], F32, tag="ps")
                        nc.tensor.matmul(ps[:, qoff:],
                                         kt4[:, ki // 4, bass.ts(ki % 4, 128)],
                                         xq4[:, h, bass.ds(c * QC + qoff, w)],
                                         start=True, stop=True)
                        et = exp_pool.tile([128, QC], F32R, tag="et")
                        nc.scalar.activation(et[:, qoff:], ps[:, qoff:], EXP,
                                             scale=SM_SCALE)
                        if j == 3:
                            nc.gpsimd.tensor_tensor(et[:, 256:], et[:, 256:],
                                                    m3_sb[:], MULT)
                        elif j >= 0:
                            lo = 128 * j
                            nc.gpsimd.tensor_tensor(
                                et[:, lo:lo + 128], et[:, lo:lo + 128],
                                tril_sb[:], MULT)
                        pending.append((st, ki, et, qoff))
                    if ki >= 4:
                        emit_acc(*pending.pop(0))
                        emit_acc(*pending.pop(0))
                    if ki == 2 and carry_in is not None:
                        carry_in[0]()
                    if ki == 3 and carry_in is not None:
                        carry_in[1]()
                    if ki >= 2 and fillers:
                        nf = max(1, (len(fillers) + nki - ki - 1) // (nki - ki))
                        for _ in range(min(nf, len(fillers))):
                            fillers.pop(0)()
                for item in pending:
                    emit_acc(*item)
                for fl in fillers:
                    fl()
                pssq = pssq_pool.tile([128, 2, QC], F32, tag="pssq",
                                      name=f"pssq{c}_{hA}")
                ars = []
                for st in range(2):
                    h = streams[st]
                    nc.vector.tensor_copy(pssq[:, st], pss[st][:])
                    ar = araw_pool.tile([128, QC], F32, tag="araw",
                                        name=f"araw{c}_{h}")
                    ars.append(ar)
                    nc.vector.tensor_copy(ar[:], pa[st][:])

                state = {}

                def fin_a():
                    lnt = rc_pool.tile([128, 2, QC], F32, tag="lnt")
                    state["lnt"] = lnt
                    nc.scalar.activation(lnt[:], pssq[:],
                                         mybir.ActivationFunctionType.Ln)

                def fin_b():
                    rc2 = rc_pool.tile([128, 2, QC], F32, tag="rc4")
                    nc.scalar.activation(rc2[:], state["lnt"][:], EXP,
                                         scale=-1.0)
                    for st in range(2):
                        at = attn_sb.tile([128, QC], BF16, tag="attnT")
                        attn_tiles[streams[st]] = at
                        nc.vector.tensor_tensor(at[:], ars[st][:],
                                                rc2[:, st], MULT)

                def fin_tail():
                    lnt = rc_pool.tile([128, 2, QC], F32, tag="lnt")
                    rc2 = rc_pool.tile([128, 2, QC], F32, tag="rc4")
                    for st in range(2):
                        nc.scalar.activation(lnt[:, st], pssq[:, st],
                                             mybir.ActivationFunctionType.Ln)
                        nc.scalar.activation(rc2[:, st], lnt[:, st], EXP,
                                             scale=-1.0)
                        at = attn_sb.tile([128, QC], BF16, tag="attnT")
                        attn_tiles[streams[st]] = at
                        nc.vector.tensor_tensor(at[:], ars[st][:],
                                                rc2[:, st], MULT)
                return fin_a, fin_b, fin_tail

            carry = None
            for c in range(NQC):
                attn_tiles = {}
                carry = emit_pair(c, 0, 1, attn_tiles, carry)
                units = []
                if prev is not None:
                    pc_, pt_ = prev
                    units = [
                        (lambda dt_=dt_: out_proj_unit(pc_, pt_, dt_))
                        for dt_ in range(NKO)
                    ]
                carry = emit_pair(c, 2, 3, attn_tiles, carry, fillers=units)
                prev = (c, attn_tiles)
            carry[2]()
            out_proj(prev[0], prev[1])

    _split_multi_waits(nc)
    return nc


def _get_state():
    if "nc" not in _CACHE:
        _CACHE["nc"] = _build_nc()
        _CACHE["consts"] = _host_consts()
    return _CACHE["nc"], _CACHE["consts"]


def kernel(data=None, mask=None, wq=None, wk=None, wv=None, wo=None, **extra):
    global LAST_RESULTS
    import ml_dtypes
    bf16 = ml_dtypes.bfloat16
    nc, consts = _get_state()

    data = np.asarray(data, dtype=np.float32)
    wq = np.asarray(wq, dtype=np.float32)
    wk = np.asarray(wk, dtype=np.float32)
    wv = np.asarray(wv, dtype=np.float32)
    wo = np.asarray(wo, dtype=np.float32)

    in_maps = []
    dTs = [np.ascontiguousarray(data[b].T).astype(bf16) for b in range(B)]
    for b in range(B):
        for g in range(NKV):
            in_maps.append({
                "dataT": dTs[b],
                "wq": wq[:, g * GQ:(g + 1) * GQ].astype(bf16),
                "wkv": np.ascontiguousarray(np.concatenate(
                    [wk[:, g * HD:(g + 1) * HD],
                     wv[:, g * HD:(g + 1) * HD]], axis=1)).astype(bf16),
                "wo": np.ascontiguousarray(wo[g * GQ:(g + 1) * GQ, :]).astype(bf16),
                "cosT": consts["cosT"],
                "sinT": consts["sinT"],
                "ctd": consts["ctd"],
                "sgn": consts["sgn"],
                "rot": consts["rot"],
                "tril": consts["tril"],
                "m3": consts["m3"],
                "ones": consts["ones"],
                "ident": consts["ident"],
            })

    res = run_bass_kernel_spmd(nc, in_maps, core_ids=list(range(8)))
    LAST_RESULTS = res

    out = np.empty((B, S, D), dtype=np.float32)
    for b in range(B):
        acc = res.results[b * NKV]["outT"].astype(np.float32).copy()
        for g in range(1, NKV):
            acc += res.results[b * NKV + g]["outT"]
        out[b] = acc.T
    return out


# revision 46
# speedup vs baseline: 1.1747x; 1.1747x over previous
import numpy as np

import bass_rust
import concourse.bass as bass
import concourse.tile as tile
import concourse.mybir as mybir
from concourse.bass_utils import run_bass_kernel_spmd

B, S, D = 2, 2048, 2048
NH, NKV, HD = 16, 4, 128
GQ = 512
NKO = D // 128
PC = 512
NPC = S // PC
QC = 512
NQC = S // QC
MAGIC = float(np.float32(12582912.0))
SM_SCALE = 1.0 / float(np.sqrt(HD))

F32 = mybir.dt.float32
F32R = mybir.dt.float32r
BF16 = mybir.dt.bfloat16
MULT = mybir.AluOpType.mult
ADD = mybir.AluOpType.add
EXP = mybir.ActivationFunctionType.Exp

_CACHE = {}

LAST_RESULTS = None


def _split_multi_waits(nc):
    for f in nc.m.functions:
        for bb in f.blocks:
            new = []
            for inst in bb.instructions:
                si = inst.sync_info
                if si is None:
                    new.append(inst)
                    continue
                waits = list(si.on_wait)
                if len(waits) > 1:
                    for k, w in enumerate(waits[:-1]):
                        nop = mybir.InstNoOp(name=f"{inst.name}-w{k}", ins=[], outs=[])
                        nop.engine = inst.engine
                        nop.sync_info = bass_rust.SyncInfo(on_wait=[w], on_update=[])
                        new.append(nop)
                    inst.sync_info = bass_rust.SyncInfo(
                        on_wait=[waits[-1]], on_update=list(si.on_update)
                    )
                new.append(inst)
            bb.instructions = new


def _host_consts():
    theta = 10000.0
    angles = 1.0 / theta ** (np.arange(0, HD, 2, dtype=np.float32) / HD)
    emb = np.outer(np.arange(S, dtype=np.float32), angles)
    emb = np.concatenate([emb, emb], axis=-1)
    cos = np.cos(emb).astype(np.float32)
    sin = np.sin(emb).astype(np.float32)
    cosT = np.ascontiguousarray(cos.T)
    sinT = np.ascontiguousarray(sin.T)

    ctd = np.ascontiguousarray(cos.reshape(S // 128, 128, HD).transpose(1, 0, 2))
    std = sin.reshape(S // 128, 128, HD).transpose(1, 0, 2).copy()
    sgn = std.copy()
    sgn[:, :, : HD // 2] = -std[:, :, : HD // 2]
    sgn = np.ascontiguousarray(sgn)

    rot = np.zeros((128, 128), dtype=np.float32)
    for i in range(64):
        rot[i, i + 64] = 1.0
        rot[i + 64, i] = -1.0

    p = np.arange(128)[:, None]
    f = np.arange(128)[None, :]
    tril = (p <= f).astype(np.float32)
    m3 = np.concatenate([np.zeros((128, 128), np.float32), tril], axis=1)

    ones = np.ones((128, 128), dtype=np.float32)
    ident = np.eye(128, dtype=np.float32)
    import ml_dtypes
    bf16 = ml_dtypes.bfloat16
    return {
        "cosT": cosT.astype(bf16), "sinT": sinT.astype(bf16),
        "ctd": ctd.astype(bf16), "sgn": sgn.astype(bf16),
        "rot": rot, "tril": tril, "m3": m3, "ones": ones, "ident": ident,
    }


def _build_nc():
    nc = bass.Bass("TRN2", target_bir_lowering=False, debug=False)

    dataT = nc.dram_tensor("dataT", [D, S], BF16, kind="ExternalInput").ap()
    wq = nc.dram_tensor("wq", [D, GQ], BF16, kind="ExternalInput").ap()
    wkv = nc.dram_tensor("wkv", [D, 2 * HD], BF16, kind="ExternalInput").ap()
    wo = nc.dram_tensor("wo", [GQ, D], BF16, kind="ExternalInput").ap()
    cosT_d = nc.dram_tensor("cosT", [128, S], BF16, kind="ExternalInput").ap()
    sinT_d = nc.dram_tensor("sinT", [128, S], BF16, kind="ExternalInput").ap()
    ctd_d = nc.dram_tensor("ctd", [128, NKO, HD], BF16, kind="ExternalInput").ap()
    sgn_d = nc.dram_tensor("sgn", [128, NKO, HD], BF16, kind="ExternalInput").ap()
    rot_d = nc.dram_tensor("rot", [128, 128], F32R, kind="ExternalInput").ap()
    tril_d = nc.dram_tensor("tril", [128, 128], F32R, kind="ExternalInput").ap()
    m3_d = nc.dram_tensor("m3", [128, 256], F32R, kind="ExternalInput").ap()
    ones_d = nc.dram_tensor("ones", [128, 128], F32R, kind="ExternalInput").ap()
    ident_d = nc.dram_tensor("ident", [128, 128], F32R, kind="ExternalInput").ap()
    outT = nc.dram_tensor("outT", [D, S], BF16, kind="ExternalOutput").ap()

    dataT_r = dataT.rearrange("(ko p) t -> p ko t", p=128)
    wq_r = wq.rearrange("(ko p) m -> p ko m", p=128)
    wkv_r = wkv.rearrange("(ko p) m -> p ko m", p=128)
    wo_r = wo.rearrange("(h p) n -> p h n", p=128)

    from contextlib import ExitStack
    with tile.TileContext(nc) as tc, ExitStack() as stack:
        small_consts = stack.enter_context(tc.tile_pool(name="sconsts", bufs=1))
        rot_sb = small_consts.tile([128, 128], F32R)
        ones_sb = small_consts.tile([128, 128], F32R)
        id_sb = small_consts.tile([128, 128], F32R)
        tril_sb = small_consts.tile([128, 128], F32R)
        m3_sb = small_consts.tile([128, 256], F32R)

        persist = stack.enter_context(tc.tile_pool(name="persist", bufs=1))
        xq4 = persist.tile([128, 4, S], F32R, name="xq4")
        kt4 = persist.tile([128, 4, QC], F32R, name="kt4")
        v_g = [persist.tile([128, 4, HD], F32R, tag=f"vg{g}", name=f"v_g{g}")
               for g in range(4)]

        GRP = 4

        with tc.tile_pool(name="p1consts", bufs=1) as p1c, \
             tc.tile_pool(name="wpool", bufs=1) as wpool, \
             tc.tile_pool(name="datapool", bufs=2) as datapool, \
             tc.tile_pool(name="kvstage", bufs=2) as kvstage, \
             tc.tile_pool(name="qtmp", bufs=2) as qtmp, \
             tc.tile_pool(name="t2pool", bufs=3) as t2pool, \
             tc.tile_pool(name="proj_ps", bufs=3, space="PSUM") as proj_ps, \
             tc.tile_pool(name="kv_ps", bufs=2, space="PSUM") as kv_ps, \
             tc.tile_pool(name="rope_ps", bufs=2, space="PSUM") as rope_ps, \
             tc.tile_pool(name="tp_ps", bufs=1, space="PSUM") as tp_ps:
            cos_sb = p1c.tile([128, S], BF16)
            sin_sb = p1c.tile([128, S], BF16)
            ctd_sb = p1c.tile([128, NKO, HD], BF16)
            sgn_sb = p1c.tile([128, NKO, HD], BF16)
            wq_sb = wpool.tile([128, NKO, GQ], BF16)
            wkv_sb = wpool.tile([128, NKO, 2 * HD], BF16)

            dT = {}
            for c in range(2):
                dT[c] = datapool.tile([128, NKO, PC], BF16, tag="dT",
                                      name=f"dT{c}")

            nc.gpsimd.dma_start(wkv_sb[:], wkv_r[:])
            nc.sync.dma_start(dT[0][:, 0:8], dataT_r[:, 0:8, bass.ts(0, PC)])
            nc.gpsimd.dma_start(dT[0][:, 8:16], dataT_r[:, 8:16, bass.ts(0, PC)])
            nc.sync.dma_start(wq_sb[:], wq_r[:])
            nc.gpsimd.dma_start(dT[1][:, 8:16], dataT_r[:, 8:16, bass.ts(1, PC)])
            nc.sync.dma_start(dT[1][:, 0:8], dataT_r[:, 0:8, bass.ts(1, PC)])
            nc.sync.dma_start(ctd_sb[:], ctd_d[:])
            nc.sync.dma_start(sgn_sb[:], sgn_d[:])
            nc.sync.dma_start(cos_sb[:], cosT_d[:])
            nc.sync.dma_start(sin_sb[:], sinT_d[:])
            nc.sync.dma_start(rot_sb[:], rot_d[:])
            nc.sync.dma_start(ones_sb[:], ones_d[:])
            nc.sync.dma_start(id_sb[:], ident_d[:])
            nc.sync.dma_start(tril_sb[:], tril_d[:])
            nc.sync.dma_start(m3_sb[:], m3_d[:])

            warm = wpool.tile([128, QC], F32, name="warm_scratch")
            nc.vector.memset(warm[:], 0.0)

            def warm_fill(n, w=QC):
                for _ in range(n):
                    wps = rope_ps.tile([128, QC], F32, tag="pr")
                    nc.tensor.matmul(wps[:, 0:w], warm[:, 0:128], warm[:, 0:w],
                                     start=True, stop=True)

            warm_fill(10)

            def quant_group(src_ap, dst_ap):
                amax = qtmp.tile([128, GRP, 1], F32, tag="amax")
                scl = qtmp.tile([128, GRP, 1], F32, tag="scl")
                inv = qtmp.tile([128, GRP, 1], F32, tag="inv")
                xs = qtmp.tile([128, GRP, HD], F32, tag="xs")
                nc.vector.tensor_reduce(amax[:], src_ap, mybir.AxisListType.X,
                                        mybir.AluOpType.max,
                                        apply_absolute_value=True)
                nc.vector.tensor_scalar_max(amax[:], amax[:], 1e-8)
                nc.vector.tensor_scalar_mul(scl[:], amax[:], 1.0 / 127.0)
                nc.vector.reciprocal(inv[:], scl[:])
                sclb = scl[:].to_broadcast((128, GRP, HD))
                invb = inv[:].to_broadcast((128, GRP, HD))
                nc.vector.tensor_tensor(xs[:], src_ap, invb, MULT)
                nc.vector.tensor_scalar_add(xs[:], xs[:], MAGIC)
                nc.vector.tensor_scalar_add(xs[:], xs[:], -MAGIC)
                nc.vector.tensor_tensor(dst_ap, xs[:], sclb, MULT)

            for c in range(NPC):
                csl = bass.ts(c, PC)
                if c + 2 < NPC:
                    cb = c + 2
                    t_ = datapool.tile([128, NKO, PC], BF16, tag="dT",
                                       name=f"dT{cb}")
                    dT[cb] = t_
                    eng = nc.gpsimd if cb % 2 else nc.sync
                    eng.dma_start(t_[:, 0:8], dataT_r[:, 0:8, bass.ts(cb, PC)])
                    eng.dma_start(t_[:, 8:16], dataT_r[:, 8:16, bass.ts(cb, PC)])

                kv_td = kvstage.tile([128, GRP, 2 * HD], F32, tag="kvtd",
                                     name=f"kvtd{c}")
                for j in range(GRP):
                    pkv = kv_ps.tile([128, 2 * HD], F32, tag="pkv")
                    for ko in range(NKO):
                        nc.tensor.matmul(pkv[:],
                                         dT[c][:, ko, bass.ds(j * 128, 128)],
                                         wkv_sb[:, ko],
                                         start=(ko == 0), stop=(ko == NKO - 1))
                    nc.scalar.copy(kv_td[:, j, :], pkv[:])

                kr = kvstage.tile([128, GRP, HD], F32, tag="kr", name=f"kr{c}")
                t2k = qtmp.tile([128, GRP, HD], F32, tag="t2k")
                tsl = bass.ts(c, GRP)
                nc.vector.tensor_tensor(kr[:], kv_td[:, :, 0:HD],
                                        ctd_sb[:, tsl], MULT)
                nc.vector.tensor_tensor(t2k[:, :, 0:64],
                                        kv_td[:, :, 64:HD],
                                        sgn_sb[:, tsl, 0:64], MULT)
                nc.vector.tensor_tensor(t2k[:, :, 64:HD],
                                        kv_td[:, :, 0:64],
                                        sgn_sb[:, tsl, 64:HD], MULT)
                nc.vector.tensor_tensor(kr[:], kr[:], t2k[:], ADD)

                kq = kvstage.tile([128, GRP, HD], F32R, tag="kq", name=f"kq{c}")
                quant_group(kr[:], kq[:])

                def emit_qproj(h):
                    pq = proj_ps.tile([128, QC], F32, tag="pq",
                                      name=f"pq{c}_{h}")
                    for ko in range(NKO):
                        nc.tensor.matmul(pq[:], wq_sb[:, ko, bass.ts(h, 128)],
                                         dT[c][:, ko],
                                         start=(ko == 0), stop=(ko == NKO - 1))
                    nc.scalar.copy(xq4[:, h, csl], pq[:])

                emit_qproj(0)
                emit_qproj(1)
                for j in range(GRP):
                    pt = tp_ps.tile([128, 128], F32R, tag="tp")
                    nc.tensor.transpose(pt[:], kq[:, j, :], id_sb[:])
                    nc.scalar.copy(kt4[:, c, bass.ts(j, 128)], pt[:])
                emit_qproj(2)
                emit_qproj(3)
                for h in range(4):
                    pr = rope_ps.tile([128, QC], F32, tag="pr")
                    nc.tensor.matmul(pr[:], rot_sb[:], xq4[:, h, csl],
                                     start=True, stop=True)
                    t1 = t2pool.tile([128, QC], F32, tag="t1")
                    t2 = t2pool.tile([128, QC], F32, tag="t2")
                    nc.vector.tensor_tensor(t1[:], xq4[:, h, csl],
                                            cos_sb[:, csl], MULT)
                    nc.vector.tensor_tensor(t2[:], pr[:], sin_sb[:, csl], MULT)
                    nc.vector.tensor_tensor(xq4[:, h, csl], t1[:], t2[:], ADD)
                quant_group(kv_td[:, :, HD:], v_g[c][:])

        with tc.tile_pool(name="p2", bufs=1) as p2, \
             tc.tile_pool(name="attn_sb", bufs=8) as attn_sb, \
             tc.tile_pool(name="exp_pool", bufs=14) as exp_pool, \
             tc.tile_pool(name="araw", bufs=6) as araw_pool, \
             tc.tile_pool(name="pssq", bufs=3) as pssq_pool, \
             tc.tile_pool(name="rc4p", bufs=2) as rc_pool, \
             tc.tile_pool(name="outstage", bufs=4) as outstage, \
             tc.tile_pool(name="score_ps", bufs=4, space="PSUM") as score_ps, \
             tc.tile_pool(name="attn_ps", bufs=2, space="PSUM") as attn_ps, \
             tc.tile_pool(name="sum_ps", bufs=2, space="PSUM") as sum_ps:
            out_ps = score_ps
            wo_t = [p2.tile([128, S], BF16, tag=f"wo{h}", name=f"wo{h}")
                    for h in range(4)]
            for h in range(4):
                nc.sync.dma_start(wo_t[h][:], wo_r[:, h])

            def out_proj_unit(c_prev, tiles, dt_):
                cpsl = bass.ts(c_prev, QC)
                po = out_ps.tile([128, QC], F32, tag="ps")
                for h2 in range(4):
                    nc.tensor.matmul(po[:], wo_t[h2][:, bass.ts(dt_, 128)],
                                     tiles[h2][:],
                                     start=(h2 == 0), stop=(h2 == 3))
                ot = outstage.tile([128, QC], BF16, tag="ot")
                nc.vector.tensor_copy(ot[:], po[:])
                eng = nc.gpsimd if dt_ % 2 else nc.sync
                eng.dma_start(outT[bass.ts(dt_, 128), cpsl], ot[:])

            def out_proj(c_prev, tiles):
                cpsl = bass.ts(c_prev, QC)
                engs = [nc.sync, nc.gpsimd, nc.scalar]
                pos = {}
                pools = [out_ps] * 4 + [attn_ps] * 2 + [sum_ps] * 2

                def finish(dt_):
                    po = pos.pop(dt_)
                    for h2 in (2, 3):
                        nc.tensor.matmul(po[:], wo_t[h2][:, bass.ts(dt_, 128)],
                                         tiles[h2][:],
                                         start=False, stop=(h2 == 3))
                    ot = outstage.tile([128, QC], BF16, tag="ot")
                    nc.vector.tensor_copy(ot[:], po[:])
                    engs[dt_ % 3].dma_start(outT[bass.ts(dt_, 128), cpsl],
                                            ot[:])

                tags = ["ps"] * 4 + ["pa"] * 2 + ["pss"] * 2
                for dt_ in range(NKO):
                    po = pools[dt_ % 8].tile([128, QC], F32, tag=tags[dt_ % 8])
                    pos[dt_] = po
                    for h2 in range(2):
                        nc.tensor.matmul(po[:], wo_t[h2][:, bass.ts(dt_, 128)],
                                         tiles[h2][:],
                                         start=(h2 == 0), stop=False)
                    if dt_ >= 7:
                        finish(dt_ - 7)
                for dt_ in range(NKO - 7, NKO):
                    finish(dt_)

            prev = None

            def emit_pair(c, hA, hB, attn_tiles, carry_in, fillers=()):
                fillers = list(fillers)
                nki = 4 * (c + 1)
                streams = (hA, hB)
                pa = [attn_ps.tile([128, QC], F32, tag="pa",
                                   name=f"pa{c}_{h}") for h in streams]
                pss = [sum_ps.tile([128, QC], F32, tag="pss",
                                   name=f"pss{c}_{h}") for h in streams]

                def emit_acc(st, ki, et, qoff):
                    nc.tensor.matmul(pss[st][:, qoff:], ones_sb[:],
                                     et[:, qoff:],
                                     start=(ki == 0), stop=(ki == nki - 1))
                    nc.tensor.matmul(pa[st][:, qoff:],
                                     v_g[ki // 4][:, ki % 4], et[:, qoff:],
                                     start=(ki == 0), stop=(ki == nki - 1))

                pending = []
                for ki in range(nki):
                    if ki >= 4 * c:
                        j = ki - 4 * c
                        qoff = min(128 * j, 256)
                    else:
                        j, qoff = -1, 0
                    w = QC - qoff
                    for st in range(2):
                        h = streams[st]
                        ps = score_ps.tile([128, QC], F32, tag="ps")
                        nc.tensor.matmul(ps[:, qoff:],
                                         kt4[:, ki // 4, bass.ts(ki % 4, 128)],
                                         xq4[:, h, bass.ds(c * QC + qoff, w)],
                                         start=True, stop=True)
                        et = exp_pool.tile([128, QC], F32R, tag="et")
                        nc.scalar.activation(et[:, qoff:], ps[:, qoff:], EXP,
                                             scale=SM_SCALE)
                        if j == 3:
                            nc.gpsimd.tensor_tensor(et[:, 256:], et[:, 256:],
                                                    m3_sb[:], MULT)
                        elif j >= 0:
                            lo = 128 * j
                            nc.gpsimd.tensor_tensor(
                                et[:, lo:lo + 128], et[:, lo:lo + 128],
                                tril_sb[:], MULT)
                        pending.append((st, ki, et, qoff))
                    if ki >= 4:
                        emit_acc(*pending.pop(0))
                        emit_acc(*pending.pop(0))
                    if ki == 2 and carry_in is not None:
                        carry_in[0]()
                    if ki == 3 and carry_in is not None:
                        carry_in[1]()
                    if ki >= 2 and fillers:
                        nf = max(1, (len(fillers) + nki - ki - 1) // (nki - ki))
                        for _ in range(min(nf, len(fillers))):
                            fillers.pop(0)()
                for item in pending:
                    emit_acc(*item)
                for fl in fillers:
                    fl()
                pssq = pssq_pool.tile([128, 2, QC], F32, tag="pssq",
                                      name=f"pssq{c}_{hA}")
                ars = []
                for st in range(2):
                    h = streams[st]
                    nc.vector.tensor_copy(pssq[:, st], pss[st][:])
                    ar = araw_pool.tile([128, QC], F32, tag="araw",
                                        name=f"araw{c}_{h}")
                    ars.append(ar)
                    nc.vector.tensor_copy(ar[:], pa[st][:])

                state = {}

                def fin_a():
                    lnt = rc_pool.tile([128, 2, QC], F32, tag="lnt")
                    state["lnt"] = lnt
                    nc.scalar.activation(lnt[:], pssq[:],
                                         mybir.ActivationFunctionType.Ln)

                def fin_b():
                    rc2 = rc_pool.tile([128, 2, QC], F32, tag="rc4")
                    nc.scalar.activation(rc2[:], state["lnt"][:], EXP,
                                         scale=-1.0)
                    for st in range(2):
                        at = attn_sb.tile([128, QC], BF16, tag="attnT")
                        attn_tiles[streams[st]] = at
                        nc.vector.tensor_tensor(at[:], ars[st][:],
                                                rc2[:, st], MULT)

                def fin_tail():
                    lnt = rc_pool.tile([128, 2, QC], F32, tag="lnt")
                    rc2 = rc_pool.tile([128, 2, QC], F32, tag="rc4")
                    for st in range(2):
                        nc.scalar.activation(lnt[:, st], pssq[:, st],
                                             mybir.ActivationFunctionType.Ln)
                        nc.scalar.activation(rc2[:, st], lnt[:, st], EXP,
                                             scale=-1.0)
                        at = attn_sb.tile([128, QC], BF16, tag="attnT")
                        attn_tiles[streams[st]] = at
                        nc.vector.tensor_tensor(at[:], ars[st][:],
                                                rc2[:, st], MULT)
                return fin_a, fin_b, fin_tail

            carry = None
            for c in range(NQC):
                attn_tiles = {}
                carry = emit_pair(c, 0, 1, attn_tiles, carry)
                units = []
                if prev is not None:
                    pc_, pt_ = prev
                    units = [
                        (lambda dt_=dt_: out_proj_unit(pc_, pt_, dt_))
                        for dt_ in range(NKO)
                    ]
                carry = emit_pair(c, 2, 3, attn_tiles, carry, fillers=units)
                prev = (c, attn_tiles)
            carry[2]()
            out_proj(prev[0], prev[1])

    _split_multi_waits(nc)
    return nc


def _get_state():
    if "nc" not in _CACHE:
        _CACHE["nc"] = _build_nc()
        _CACHE["consts"] = _host_consts()
    return _CACHE["nc"], _CACHE["consts"]


def kernel(data=None, mask=None, wq=None, wk=None, wv=None, wo=None, **extra):
    global LAST_RESULTS
    import ml_dtypes
    bf16 = ml_dtypes.bfloat16
    nc, consts = _get_state()

    data = np.asarray(data, dtype=np.float32)
    wq = np.asarray(wq, dtype=np.float32)
    wk = np.asarray(wk, dtype=np.float32)
    wv = np.asarray(wv, dtype=np.float32)
    wo = np.asarray(wo, dtype=np.float32)

    in_maps = []
    dTs = [np.ascontiguousarray(data[b].T).astype(bf16) for b in range(B)]
    for b in range(B):
        for g in range(NKV):
            in_maps.append({
                "dataT": dTs[b],
                "wq": wq[:, g * GQ:(g + 1) * GQ].astype(bf16),
                "wkv": np.ascontiguousarray(np.concatenate(
                    [wk[:, g * HD:(g + 1) * HD],
                     wv[:, g * HD:(g + 1) * HD]], axis=1)).astype(bf16),
                "wo": np.ascontiguousarray(wo[g * GQ:(g + 1) * GQ, :]).astype(bf16),
                "cosT": consts["cosT"],
                "sinT": consts["sinT"],
                "ctd": consts["ctd"],
                "sgn": consts["sgn"],
                "rot": consts["rot"],
                "tril": consts["tril"],
                "m3": consts["m3"],
                "ones": consts["ones"],
                "ident": consts["ident"],
            })

    res = run_bass_kernel_spmd(nc, in_maps, core_ids=list(range(8)))
    LAST_RESULTS = res

    out = np.empty((B, S, D), dtype=np.float32)
    for b in range(B):
        acc = res.results[b * NKV]["outT"].astype(np.float32).copy()
        for g in range(1, NKV):
            acc += res.results[b * NKV + g]["outT"]
        out[b] = acc.T
    return out


# revision 47
# speedup vs baseline: 1.1835x; 1.0075x over previous
import numpy as np

import bass_rust
import concourse.bass as bass
import concourse.tile as tile
import concourse.mybir as mybir
from concourse.bass_utils import run_bass_kernel_spmd

B, S, D = 2, 2048, 2048
NH, NKV, HD = 16, 4, 128
GQ = 512
NKO = D // 128
PC = 512
NPC = S // PC
QC = 512
NQC = S // QC
MAGIC = float(np.float32(12582912.0))
SM_SCALE = 1.0 / float(np.sqrt(HD))

F32 = mybir.dt.float32
F32R = mybir.dt.float32r
BF16 = mybir.dt.bfloat16
MULT = mybir.AluOpType.mult
ADD = mybir.AluOpType.add
EXP = mybir.ActivationFunctionType.Exp

_CACHE = {}

LAST_RESULTS = None


def _split_multi_waits(nc):
    for f in nc.m.functions:
        for bb in f.blocks:
            new = []
            for inst in bb.instructions:
                si = inst.sync_info
                if si is None:
                    new.append(inst)
                    continue
                waits = list(si.on_wait)
                if len(waits) > 1:
                    for k, w in enumerate(waits[:-1]):
                        nop = mybir.InstNoOp(name=f"{inst.name}-w{k}", ins=[], outs=[])
                        nop.engine = inst.engine
                        nop.sync_info = bass_rust.SyncInfo(on_wait=[w], on_update=[])
                        new.append(nop)
                    inst.sync_info = bass_rust.SyncInfo(
                        on_wait=[waits[-1]], on_update=list(si.on_update)
                    )
                new.append(inst)
            bb.instructions = new


def _host_consts():
    theta = 10000.0
    angles = 1.0 / theta ** (np.arange(0, HD, 2, dtype=np.float32) / HD)
    emb = np.outer(np.arange(S, dtype=np.float32), angles)
    emb = np.concatenate([emb, emb], axis=-1)
    cos = np.cos(emb).astype(np.float32)
    sin = np.sin(emb).astype(np.float32)
    cosT = np.ascontiguousarray(cos.T)
    sinT = np.ascontiguousarray(sin.T)

    ctd = np.ascontiguousarray(cos.reshape(S // 128, 128, HD).transpose(1, 0, 2))
    std = sin.reshape(S // 128, 128, HD).transpose(1, 0, 2).copy()
    sgn = std.copy()
    sgn[:, :, : HD // 2] = -std[:, :, : HD // 2]
    sgn = np.ascontiguousarray(sgn)

    rot = np.zeros((128, 128), dtype=np.float32)
    for i in range(64):
        rot[i, i + 64] = 1.0
        rot[i + 64, i] = -1.0

    p = np.arange(128)[:, None]
    f = np.arange(128)[None, :]
    tril = (p <= f).astype(np.float32)
    m3 = np.concatenate([np.zeros((128, 128), np.float32), tril], axis=1)

    ones = np.ones((128, 128), dtype=np.float32)
    ident = np.eye(128, dtype=np.float32)
    import ml_dtypes
    bf16 = ml_dtypes.bfloat16
    return {
        "cosT": cosT.astype(bf16), "sinT": sinT.astype(bf16),
        "ctd": ctd.astype(bf16), "sgn": sgn.astype(bf16),
        "rot": rot, "tril": tril, "m3": m3, "ones": ones, "ident": ident,
    }


def _build_nc():
    nc = bass.Bass("TRN2", target_bir_lowering=False, debug=False)

    dataT = nc.dram_tensor("dataT", [D, S], BF16, kind="ExternalInput").ap()
    wq = nc.dram_tensor("wq", [D, GQ], BF16, kind="ExternalInput").ap()
    wkv = nc.dram_tensor("wkv", [D, 2 * HD], BF16, kind="ExternalInput").ap()
    wo = nc.dram_tensor("wo", [GQ, D], BF16, kind="ExternalInput").ap()
    cosT_d = nc.dram_tensor("cosT", [128, S], BF16, kind="ExternalInput").ap()
    sinT_d = nc.dram_tensor("sinT", [128, S], BF16, kind="ExternalInput").ap()
    ctd_d = nc.dram_tensor("ctd", [128, NKO, HD], BF16, kind="ExternalInput").ap()
    sgn_d = nc.dram_tensor("sgn", [128, NKO, HD], BF16, kind="ExternalInput").ap()
    rot_d = nc.dram_tensor("rot", [128, 128], F32R, kind="ExternalInput").ap()
    tril_d = nc.dram_tensor("tril", [128, 128], F32R, kind="ExternalInput").ap()
    m3_d = nc.dram_tensor("m3", [128, 256], F32R, kind="ExternalInput").ap()
    ones_d = nc.dram_tensor("ones", [128, 128], F32R, kind="ExternalInput").ap()
    ident_d = nc.dram_tensor("ident", [128, 128], F32R, kind="ExternalInput").ap()
    outT = nc.dram_tensor("outT", [D, S], BF16, kind="ExternalOutput").ap()

    dataT_r = dataT.rearrange("(ko p) t -> p ko t", p=128)
    wq_r = wq.rearrange("(ko p) m -> p ko m", p=128)
    wkv_r = wkv.rearrange("(ko p) m -> p ko m", p=128)
    wo_r = wo.rearrange("(h p) n -> p h n", p=128)

    from contextlib import ExitStack
    with tile.TileContext(nc) as tc, ExitStack() as stack:
        small_consts = stack.enter_context(tc.tile_pool(name="sconsts", bufs=1))
        rot_sb = small_consts.tile([128, 128], F32R)
        ones_sb = small_consts.tile([128, 128], F32R)
        id_sb = small_consts.tile([128, 128], F32R)
        tril_sb = small_consts.tile([128, 128], F32R)
        m3_sb = small_consts.tile([128, 256], F32R)

        persist = stack.enter_context(tc.tile_pool(name="persist", bufs=1))
        xq4 = persist.tile([128, 4, S], F32R, name="xq4")
        kt4 = persist.tile([128, 4, QC], F32R, name="kt4")
        v_g = [persist.tile([128, 4, HD], F32R, tag=f"vg{g}", name=f"v_g{g}")
               for g in range(4)]

        GRP = 4

        with tc.tile_pool(name="p1consts", bufs=1) as p1c, \
             tc.tile_pool(name="wpool", bufs=1) as wpool, \
             tc.tile_pool(name="datapool", bufs=2) as datapool, \
             tc.tile_pool(name="kvstage", bufs=2) as kvstage, \
             tc.tile_pool(name="qtmp", bufs=2) as qtmp, \
             tc.tile_pool(name="t2pool", bufs=3) as t2pool, \
             tc.tile_pool(name="proj_ps", bufs=3, space="PSUM") as proj_ps, \
             tc.tile_pool(name="kv_ps", bufs=2, space="PSUM") as kv_ps, \
             tc.tile_pool(name="rope_ps", bufs=2, space="PSUM") as rope_ps, \
             tc.tile_pool(name="tp_ps", bufs=1, space="PSUM") as tp_ps:
            cos_sb = p1c.tile([128, S], BF16)
            sin_sb = p1c.tile([128, S], BF16)
            ctd_sb = p1c.tile([128, NKO, HD], BF16)
            sgn_sb = p1c.tile([128, NKO, HD], BF16)
            wq_sb = wpool.tile([128, NKO, GQ], BF16)
            wkv_sb = wpool.tile([128, NKO, 2 * HD], BF16)

            dT = {}
            for c in range(2):
                dT[c] = datapool.tile([128, NKO, PC], BF16, tag="dT",
                                      name=f"dT{c}")

            nc.gpsimd.dma_start(wkv_sb[:], wkv_r[:])
            nc.sync.dma_start(dT[0][:, 0:8], dataT_r[:, 0:8, bass.ts(0, PC)])
            nc.gpsimd.dma_start(dT[0][:, 8:16], dataT_r[:, 8:16, bass.ts(0, PC)])
            nc.sync.dma_start(wq_sb[:], wq_r[:])
            nc.gpsimd.dma_start(dT[1][:, 8:16], dataT_r[:, 8:16, bass.ts(1, PC)])
            nc.sync.dma_start(dT[1][:, 0:8], dataT_r[:, 0:8, bass.ts(1, PC)])
            nc.sync.dma_start(ctd_sb[:], ctd_d[:])
            nc.sync.dma_start(sgn_sb[:], sgn_d[:])
            nc.sync.dma_start(cos_sb[:], cosT_d[:])
            nc.sync.dma_start(sin_sb[:], sinT_d[:])
            nc.sync.dma_start(rot_sb[:], rot_d[:])
            nc.sync.dma_start(ones_sb[:], ones_d[:])
            nc.sync.dma_start(id_sb[:], ident_d[:])
            nc.sync.dma_start(tril_sb[:], tril_d[:])
            nc.sync.dma_start(m3_sb[:], m3_d[:])

            warm = wpool.tile([128, QC], F32, name="warm_scratch")
            nc.vector.memset(warm[:], 0.0)

            def warm_fill(n, w=QC):
                for _ in range(n):
                    wps = rope_ps.tile([128, QC], F32, tag="pr")
                    nc.tensor.matmul(wps[:, 0:w], warm[:, 0:128], warm[:, 0:w],
                                     start=True, stop=True)

            warm_fill(7)

            def quant_group(src_ap, dst_ap):
                amax = qtmp.tile([128, GRP, 1], F32, tag="amax")
                scl = qtmp.tile([128, GRP, 1], F32, tag="scl")
                inv = qtmp.tile([128, GRP, 1], F32, tag="inv")
                xs = qtmp.tile([128, GRP, HD], F32, tag="xs")
                nc.vector.tensor_reduce(amax[:], src_ap, mybir.AxisListType.X,
                                        mybir.AluOpType.max,
                                        apply_absolute_value=True)
                nc.vector.tensor_scalar_max(amax[:], amax[:], 1e-8)
                nc.vector.tensor_scalar_mul(scl[:], amax[:], 1.0 / 127.0)
                nc.vector.reciprocal(inv[:], scl[:])
                sclb = scl[:].to_broadcast((128, GRP, HD))
                invb = inv[:].to_broadcast((128, GRP, HD))
                nc.vector.tensor_tensor(xs[:], src_ap, invb, MULT)
                nc.vector.tensor_scalar_add(xs[:], xs[:], MAGIC)
                nc.vector.tensor_scalar_add(xs[:], xs[:], -MAGIC)
                nc.vector.tensor_tensor(dst_ap, xs[:], sclb, MULT)

            for c in range(NPC):
                csl = bass.ts(c, PC)
                if c + 2 < NPC:
                    cb = c + 2
                    t_ = datapool.tile([128, NKO, PC], BF16, tag="dT",
                                       name=f"dT{cb}")
                    dT[cb] = t_
                    eng = nc.gpsimd if cb % 2 else nc.sync
                    eng.dma_start(t_[:, 0:8], dataT_r[:, 0:8, bass.ts(cb, PC)])
                    eng.dma_start(t_[:, 8:16], dataT_r[:, 8:16, bass.ts(cb, PC)])

                kv_td = kvstage.tile([128, GRP, 2 * HD], F32, tag="kvtd",
                                     name=f"kvtd{c}")
                for j in range(GRP):
                    pkv = kv_ps.tile([128, 2 * HD], F32, tag="pkv")
                    for ko in range(NKO):
                        nc.tensor.matmul(pkv[:],
                                         dT[c][:, ko, bass.ds(j * 128, 128)],
                                         wkv_sb[:, ko],
                                         start=(ko == 0), stop=(ko == NKO - 1))
                    nc.scalar.copy(kv_td[:, j, :], pkv[:])

                kr = kvstage.tile([128, GRP, HD], F32, tag="kr", name=f"kr{c}")
                t2k = qtmp.tile([128, GRP, HD], F32, tag="t2k")
                tsl = bass.ts(c, GRP)
                nc.vector.tensor_tensor(kr[:], kv_td[:, :, 0:HD],
                                        ctd_sb[:, tsl], MULT)
                nc.vector.tensor_tensor(t2k[:, :, 0:64],
                                        kv_td[:, :, 64:HD],
                                        sgn_sb[:, tsl, 0:64], MULT)
                nc.vector.tensor_tensor(t2k[:, :, 64:HD],
                                        kv_td[:, :, 0:64],
                                        sgn_sb[:, tsl, 64:HD], MULT)
                nc.vector.tensor_tensor(kr[:], kr[:], t2k[:], ADD)

                kq = kvstage.tile([128, GRP, HD], F32R, tag="kq", name=f"kq{c}")
                quant_group(kr[:], kq[:])

                def emit_qproj(h):
                    pq = proj_ps.tile([128, QC], F32, tag="pq",
                                      name=f"pq{c}_{h}")
                    for ko in range(NKO):
                        nc.tensor.matmul(pq[:], wq_sb[:, ko, bass.ts(h, 128)],
                                         dT[c][:, ko],
                                         start=(ko == 0), stop=(ko == NKO - 1))
                    nc.scalar.copy(xq4[:, h, csl], pq[:])

                emit_qproj(0)
                emit_qproj(1)
                for j in range(GRP):
                    pt = tp_ps.tile([128, 128], F32R, tag="tp")
                    nc.tensor.transpose(pt[:], kq[:, j, :], id_sb[:])
                    nc.scalar.copy(kt4[:, c, bass.ts(j, 128)], pt[:])
                emit_qproj(2)
                emit_qproj(3)
                for h in range(4):
                    pr = rope_ps.tile([128, QC], F32, tag="pr")
                    nc.tensor.matmul(pr[:], rot_sb[:], xq4[:, h, csl],
                                     start=True, stop=True)
                    t1 = t2pool.tile([128, QC], F32, tag="t1")
                    t2 = t2pool.tile([128, QC], F32, tag="t2")
                    nc.vector.tensor_tensor(t1[:], xq4[:, h, csl],
                                            cos_sb[:, csl], MULT)
                    nc.vector.tensor_tensor(t2[:], pr[:], sin_sb[:, csl], MULT)
                    nc.vector.tensor_tensor(xq4[:, h, csl], t1[:], t2[:], ADD)
                quant_group(kv_td[:, :, HD:], v_g[c][:])

        with tc.tile_pool(name="p2", bufs=1) as p2, \
             tc.tile_pool(name="attn_sb", bufs=8) as attn_sb, \
             tc.tile_pool(name="exp_pool", bufs=14) as exp_pool, \
             tc.tile_pool(name="araw", bufs=6) as araw_pool, \
             tc.tile_pool(name="pssq", bufs=3) as pssq_pool, \
             tc.tile_pool(name="rc4p", bufs=2) as rc_pool, \
             tc.tile_pool(name="outstage", bufs=4) as outstage, \
             tc.tile_pool(name="score_ps", bufs=4, space="PSUM") as score_ps, \
             tc.tile_pool(name="attn_ps", bufs=2, space="PSUM") as attn_ps, \
             tc.tile_pool(name="sum_ps", bufs=2, space="PSUM") as sum_ps:
            out_ps = score_ps
            wo_t = [p2.tile([128, S], BF16, tag=f"wo{h}", name=f"wo{h}")
                    for h in range(4)]
            for h in range(4):
                nc.sync.dma_start(wo_t[h][:], wo_r[:, h])

            def out_proj_unit(c_prev, tiles, dt_):
                cpsl = bass.ts(c_prev, QC)
                po = out_ps.tile([128, QC], F32, tag="ps")
                for h2 in range(4):
                    nc.tensor.matmul(po[:], wo_t[h2][:, bass.ts(dt_, 128)],
                                     tiles[h2][:],
                                     start=(h2 == 0), stop=(h2 == 3))
                ot = outstage.tile([128, QC], BF16, tag="ot")
                nc.vector.tensor_copy(ot[:], po[:])
                eng = nc.gpsimd if dt_ % 2 else nc.sync
                eng.dma_start(outT[bass.ts(dt_, 128), cpsl], ot[:])

            def out_proj(c_prev, tiles):
                cpsl = bass.ts(c_prev, QC)
                engs = [nc.sync, nc.gpsimd, nc.scalar]
                pos = {}
                pools = [out_ps] * 4 + [attn_ps] * 2 + [sum_ps] * 2

                def finish(dt_):
                    po = pos.pop(dt_)
                    for h2 in (2, 3):
                        nc.tensor.matmul(po[:], wo_t[h2][:, bass.ts(dt_, 128)],
                                         tiles[h2][:],
                                         start=False, stop=(h2 == 3))
                    ot = outstage.tile([128, QC], BF16, tag="ot")
                    nc.vector.tensor_copy(ot[:], po[:])
                    engs[dt_ % 3].dma_start(outT[bass.ts(dt_, 128), cpsl],
                                            ot[:])

                tags = ["ps"] * 4 + ["pa"] * 2 + ["pss"] * 2
                for dt_ in range(NKO):
                    po = pools[dt_ % 8].tile([128, QC], F32, tag=tags[dt_ % 8])
                    pos[dt_] = po
                    for h2 in range(2):
                        nc.tensor.matmul(po[:], wo_t[h2][:, bass.ts(dt_, 128)],
                                         tiles[h2][:],
                                         start=(h2 == 0), stop=False)
                    if dt_ >= 7:
                        finish(dt_ - 7)
                for dt_ in range(NKO - 7, NKO):
                    finish(dt_)

            prev = None

            def emit_pair(c, hA, hB, attn_tiles, carry_in, fillers=()):
                fillers = list(fillers)
                nki = 4 * (c + 1)
                streams = (hA, hB)
                pa = [attn_ps.tile([128, QC], F32, tag="pa",
                                   name=f"pa{c}_{h}") for h in streams]
                pss = [sum_ps.tile([128, QC], F32, tag="pss",
                                   name=f"pss{c}_{h}") for h in streams]

                def emit_acc(st, ki, et, qoff):
                    nc.tensor.matmul(pss[st][:, qoff:], ones_sb[:],
                                     et[:, qoff:],
                                     start=(ki == 0), stop=(ki == nki - 1))
                    nc.tensor.matmul(pa[st][:, qoff:],
                                     v_g[ki // 4][:, ki % 4], et[:, qoff:],
                                     start=(ki == 0), stop=(ki == nki - 1))

                pending = []
                for ki in range(nki):
                    if ki >= 4 * c:
                        j = ki - 4 * c
                        qoff = min(128 * j, 256)
                    else:
                        j, qoff = -1, 0
                    w = QC - qoff
                    for st in range(2):
                        h = streams[st]
                        ps = score_ps.tile([128, QC], F32, tag="ps")
                        nc.tensor.matmul(ps[:, qoff:],
                                         kt4[:, ki // 4, bass.ts(ki % 4, 128)],
                                         xq4[:, h, bass.ds(c * QC + qoff, w)],
                                         start=True, stop=True)
                        et = exp_pool.tile([128, QC], F32R, tag="et")
                        nc.scalar.activation(et[:, qoff:], ps[:, qoff:], EXP,
                                             scale=SM_SCALE)
                        if j == 3:
                            nc.gpsimd.tensor_tensor(et[:, 256:], et[:, 256:],
                                                    m3_sb[:], MULT)
                        elif j >= 0:
                            lo = 128 * j
                            nc.gpsimd.tensor_tensor(
                                et[:, lo:lo + 128], et[:, lo:lo + 128],
                                tril_sb[:], MULT)
                        pending.append((st, ki, et, qoff))
                    if ki >= 4:
                        emit_acc(*pending.pop(0))
                        emit_acc(*pending.pop(0))
                    if ki == 2 and carry_in is not None:
                        carry_in[0]()
                    if ki == 3 and carry_in is not None:
                        carry_in[1]()
                    if ki >= 2 and fillers:
                        nf = max(1, (len(fillers) + nki - ki - 1) // (nki - ki))
                        for _ in range(min(nf, len(fillers))):
                            fillers.pop(0)()
                for item in pending:
                    emit_acc(*item)
                for fl in fillers:
                    fl()
                pssq = pssq_pool.tile([128, 2, QC], F32, tag="pssq",
                                      name=f"pssq{c}_{hA}")
                ars = []
                for st in range(2):
                    h = streams[st]
                    nc.vector.tensor_copy(pssq[:, st], pss[st][:])
                    ar = araw_pool.tile([128, QC], F32, tag="araw",
                                        name=f"araw{c}_{h}")
                    ars.append(ar)
                    nc.vector.tensor_copy(ar[:], pa[st][:])

                state = {}

                def fin_a():
                    lnt = rc_pool.tile([128, 2, QC], F32, tag="lnt")
                    state["lnt"] = lnt
                    nc.scalar.activation(lnt[:], pssq[:],
                                         mybir.ActivationFunctionType.Ln)

                def fin_b():
                    rc2 = rc_pool.tile([128, 2, QC], F32, tag="rc4")
                    nc.scalar.activation(rc2[:], state["lnt"][:], EXP,
                                         scale=-1.0)
                    for st in range(2):
                        at = attn_sb.tile([128, QC], BF16, tag="attnT")
                        attn_tiles[streams[st]] = at
                        nc.vector.tensor_tensor(at[:], ars[st][:],
                                                rc2[:, st], MULT)

                def fin_tail():
                    lnt = rc_pool.tile([128, 2, QC], F32, tag="lnt")
                    rc2 = rc_pool.tile([128, 2, QC], F32, tag="rc4")
                    for st in range(2):
                        nc.scalar.activation(lnt[:, st], pssq[:, st],
                                             mybir.ActivationFunctionType.Ln)
                        nc.scalar.activation(rc2[:, st], lnt[:, st], EXP,
                                             scale=-1.0)
                        at = attn_sb.tile([128, QC], BF16, tag="attnT")
                        attn_tiles[streams[st]] = at
                        nc.vector.tensor_tensor(at[:], ars[st][:],
                                                rc2[:, st], MULT)
                return fin_a, fin_b, fin_tail

            carry = None
            for c in range(NQC):
                attn_tiles = {}
                carry = emit_pair(c, 0, 1, attn_tiles, carry)
                units = []
                if prev is not None:
                    pc_, pt_ = prev
                    units = [
                        (lambda dt_=dt_: out_proj_unit(pc_, pt_, dt_))
                        for dt_ in range(NKO)
                    ]
                carry = emit_pair(c, 2, 3, attn_tiles, carry, fillers=units)
                prev = (c, attn_tiles)
            carry[2]()
            out_proj(prev[0], prev[1])

    _split_multi_waits(nc)
    return nc


def _get_state():
    if "nc" not in _CACHE:
        _CACHE["nc"] = _build_nc()
        _CACHE["consts"] = _host_consts()
    return _CACHE["nc"], _CACHE["consts"]


def kernel(data=None, mask=None, wq=None, wk=None, wv=None, wo=None, **extra):
    global LAST_RESULTS
    import ml_dtypes
    bf16 = ml_dtypes.bfloat16
    nc, consts = _get_state()

    data = np.asarray(data, dtype=np.float32)
    wq = np.asarray(wq, dtype=np.float32)
    wk = np.asarray(wk, dtype=np.float32)
    wv = np.asarray(wv, dtype=np.float32)
    wo = np.asarray(wo, dtype=np.float32)

    in_maps = []
    dTs = [np.ascontiguousarray(data[b].T).astype(bf16) for b in range(B)]
    for b in range(B):
        for g in range(NKV):
            in_maps.append({
                "dataT": dTs[b],
                "wq": wq[:, g * GQ:(g + 1) * GQ].astype(bf16),
                "wkv": np.ascontiguousarray(np.concatenate(
                    [wk[:, g * HD:(g + 1) * HD],
                     wv[:, g * HD:(g + 1) * HD]], axis=1)).astype(bf16),
                "wo": np.ascontiguousarray(wo[g * GQ:(g + 1) * GQ, :]).astype(bf16),
                "cosT": consts["cosT"],
                "sinT": consts["sinT"],
                "ctd": consts["ctd"],
                "sgn": consts["sgn"],
                "rot": consts["rot"],
                "tril": consts["tril"],
                "m3": consts["m3"],
                "ones": consts["ones"],
                "ident": consts["ident"],
            })

    res = run_bass_kernel_spmd(nc, in_maps, core_ids=list(range(8)))
    LAST_RESULTS = res

    out = np.empty((B, S, D), dtype=np.float32)
    for b in range(B):
        acc = res.results[b * NKV]["outT"].astype(np.float32).copy()
        for g in range(1, NKV):
            acc += res.results[b * NKV + g]["outT"]
        out[b] = acc.T
    return out
